# revision 40
# baseline (speedup 1.0000x reference)
"""Single-head attention (B=4, L=4096, EMB=312, HID=256) on 8 NeuronCores.

Sharding: data-parallel over batch (4) x key-parallel (2) = 8 cores. Each
core handles ALL 4096 queries against its half of the keys and returns the
UNNORMALIZED partial [sum_k p*v | sum_k p] rows; the host combines the two
halves as (o1+o2)/(s1+s2).

To keep a single SPMD program, the host permutes each core's query columns
so the core's key half is always columns 0..2047 of its embT (the mask
columns and output rows are permuted identically; the host inverts the
permutation when combining).

Per-core device algorithm (everything single-pass):
  - embT/weights are fp32 in DRAM, tagged float32r: fp32r matmuls with
    free-dim >= 256 run at the bf16 PE rate, so projections, QK and the
    identity-free score pipeline all run 1 cycle/row.
  - embT carries a ones-row at index EMB and W* carry the bias in that row,
    so projections fold the bias in. Wv has 2 extra columns: ones (gives the
    softmax row-sum through the P@V matmul) and zero padding (even N).
  - Scores are computed transposed: sT[kl, ql] = kT-chunk^T @ qT. exp() runs
    directly on the score PSUM (no mask pre-add) writing bf16; the binary
    {0,1} bf16 mask is applied MULTIPLICATIVELY post-exp on the DVE, which
    hits the 2x packed-16-bit perf mode. The masked bf16 probabilities are
    the stationary operand of the bf16 P@V matmul.
  - PSUM->SBUF copies split across DVE (k/v) and ACT (early q) — GPSIMD
    cannot access PSUM. The mask is two half-tile DMAs per query tile
    ([1024 x 512] bf16 each) and the output one DMA per query-tile pair,
    keeping the serial HWDGE ring (~630ns/descriptor-gen) cold. Only the
    313 real embT/weight rows travel over DMA; the zero tail of the third
    partition-chunk is memset once.
  - P@V for chunk kc is emitted LOOK=6 chunks behind its QK so the PE
    always has independent work while ACT exp + DVE mask-mult are in
    flight, and the qt-boundary PSUM-bank WAR release is off the PE
    critical path. A ~4us zero-matmul warmup stream keeps the PE p-state
    ramp warm through the startup DMA wait.

  NOTE (scheduler pitfall, found via CoreSim): emitting a mask prefetch
  DMA *between* two embT block loads made the tile scheduler drop the
  DMA-completion wait on the later block's consumer matmuls (silent data
  race on HW). All mask prefetches are emitted strictly after the embT
  block loads of the same program region.
"""
import os

import numpy as np
import ml_dtypes

import concourse.bacc as bacc
import concourse.tile as tile
from concourse import mybir, bass2jax
from concourse.bass_utils import run_bass_kernel_spmd

# Debug aid (opt-in): surface real compile errors from the PJRT compile
# hook, which the C++ bridge otherwise swallows.
if os.environ.get("BASS_KERNEL_DEBUG"):
    import functools as _ft
    import traceback as _tb
    _orig_hook = bass2jax.neuronx_cc_hook
    @_ft.wraps(_orig_hook)
    def _dbg_hook(*args, **kwargs):
        try:
            return _orig_hook(*args, **kwargs)
        except BaseException:
            _tb.print_exc()
            raise
    bass2jax.neuronx_cc_hook = _dbg_hook

EMB, HID, B, L = 312, 256, 4, 4096
NCORES = 8
P = 128
KL = L // 2            # key rows per core (key-parallel halves)
EPAD = 384             # emb dim padded to 3 partition chunks; row EMB is the ones-row
HV = HID + 2           # v columns: HID values | ones | zero pad (even N for matmul)
QT = 512               # ql tile width (PSUM bank = 512 fp32)
NKC = KL // P          # 16 kl chunks per core
NQTT = L // QT         # 8 ql tiles per core (all queries)
NKT = KL // QT         # 4 l tiles for the k projection
LOOK = 6               # P@V emission lag (chunks) behind QK

F32 = mybir.dt.float32
F32R = mybir.dt.float32r
BF16 = mybir.dt.bfloat16
BF = ml_dtypes.bfloat16

_CACHE = {}


def _build():
    nc = bacc.Bacc(None)

    ER = EMB + 1
    E2 = ER - 2 * P
    embT_d = nc.dram_tensor("embT", [ER, L], F32R, kind="ExternalInput")
    wq_d = nc.dram_tensor("wq", [ER, HID], F32R, kind="ExternalInput")
    wk_d = nc.dram_tensor("wk", [ER, HID], F32R, kind="ExternalInput")
    wv_d = nc.dram_tensor("wv", [ER, HV], F32R, kind="ExternalInput")
    maskT_d = nc.dram_tensor("maskT", [KL, L], BF16, kind="ExternalInput")
    out_d = nc.dram_tensor("out", [L, HID + 1], F32, kind="ExternalOutput")

    with tile.TileContext(nc) as tc:
        with (
            tc.tile_pool(name="big", bufs=1) as big,
            tc.tile_pool(name="mtp", bufs=2) as mtp,
            tc.tile_pool(name="pep", bufs=3) as pep,
            tc.tile_pool(name="ptp", bufs=9) as ptp,
            tc.tile_pool(name="fin", bufs=2) as fin,
            tc.tile_pool(name="ps_st", bufs=4, space="PSUM") as ps_st,
            tc.tile_pool(name="ps_pv", bufs=1, space="PSUM") as ps_pv,
        ):
            def cpn(d):
                return d.rearrange("(c p) n -> p c n", p=P)

            # ---- PE warmup during the startup DMA wait
            warm = big.tile([P, QT], F32R, name="warm")
            nc.vector.memset(warm.bitcast(F32), 0.0)
            wps = ps_st.tile([P, QT], F32, name="st", tag="st")
            for _ in range(10):
                nc.tensor.matmul(wps, lhsT=warm[:, :P], rhs=warm,
                                 start=True, stop=True)

            # ---- startup DMAs (trimmed rows, per-chunk writes)
            embT_t = big.tile([P, 3, L], F32R, name="embT_t")
            wk_t = big.tile([P, 3, HID], F32R, name="wk_t")
            wv_t = big.tile([P, 3, HV], F32R, name="wv_t")
            wq_t = big.tile([P, 3, HID], F32R, name="wq_t")
            for wt in (wk_t, wv_t, wq_t):
                nc.gpsimd.memset(wt[:, 2, :].bitcast(F32), 0.0)
            nc.gpsimd.memset(embT_t[:, 2, :].bitcast(F32), 0.0)

            def load_trim(dst3, dram, csl):
                nc.sync.dma_start(
                    out=dst3[:, 0:2, :],
                    in_=dram[0:2 * P, csl].rearrange("(c p) n -> p c n", p=P))
                nc.sync.dma_start(out=dst3[0:E2, 2, :], in_=dram[2 * P:ER, csl])

            def load_block(b):
                sl = slice(b * QT, (b + 1) * QT)
                load_trim(embT_t[:, :, sl], embT_d, sl)

            load_trim(wk_t[:, :, :P], wk_d, slice(0, P))
            load_trim(embT_t[:, :, 0:QT // 2], embT_d, slice(0, QT // 2))
            load_trim(wk_t[:, :, P:], wk_d, slice(P, HID))
            load_trim(embT_t[:, :, QT // 2:QT], embT_d, slice(QT // 2, QT))
            load_trim(wv_t, wv_d, slice(0, HV))
            load_block(1)
            load_trim(wq_t, wq_d, slice(0, HID))
            load_block(2)
            load_block(3)

            # ---- projection destinations
            kT_t = big.tile([P, 2, KL], F32R, name="kT_t")
            qT_t = big.tile([P, 2, L], F32R, name="qT_t")
            v_t = big.tile([P, NKC, HV], BF16, name="v_t")

            def emit_kq(which, hc, c0, cw=QT):
                ps = ps_st.tile([P, QT], F32, name="st", tag="st")
                w, dstT = (wk_t, kT_t) if which == "k" else (wq_t, qT_t)
                lsl = slice(c0, c0 + cw)
                for e in range(3):
                    nc.tensor.matmul(
                        ps[:, :cw],
                        lhsT=w[:, e, hc * P:(hc + 1) * P],
                        rhs=embT_t[:, e, lsl],
                        start=(e == 0), stop=(e == 2),
                    )
                if which == "q" and c0 < NKT * QT:
                    # early q copies ride ACT (idle until the first exp);
                    # in-attention ones go to DVE to keep the exp stream tight
                    nc.scalar.copy(dstT[:, hc, lsl], ps[:, :cw])
                else:
                    nc.vector.tensor_copy(dstT[:, hc, lsl], ps[:, :cw])

            def emit_v(kc):
                ps = ps_pv.tile([P, HV], F32, name="vps", tag=f"pv{kc % 4}")
                for e in range(3):
                    nc.tensor.matmul(
                        ps,
                        lhsT=embT_t[:, e, kc * P:(kc + 1) * P],
                        rhs=wv_t[:, e, :],
                        start=(e == 0), stop=(e == 2),
                    )
                nc.vector.tensor_copy(v_t[:, kc, :], ps)

            def warm_fill(n):
                for _ in range(n):
                    nc.tensor.matmul(wps, lhsT=warm[:, :P], rhs=warm,
                                     start=True, stop=True)

            emit_kq("k", 0, 0, QT // 2)
            emit_kq("k", 1, 0, QT // 2)
            emit_kq("k", 0, QT // 2, QT // 2)
            emit_kq("k", 1, QT // 2, QT // 2)
            warm_fill(3)
            for kc in range(4):
                emit_v(kc)
            warm_fill(3)
            for lt in range(1, NKT):
                emit_kq("k", 0, lt * QT)
                emit_kq("k", 1, lt * QT)
                if lt == 1:
                    warm_fill(2)
                for kc in range(4 * lt, 4 * lt + 4):
                    emit_v(kc)
            # q projections for the first half of the query tiles
            for lt in range(NKT):
                emit_kq("q", 0, lt * QT)
                emit_kq("q", 1, lt * QT)

            # ---- attention
            mask_tiles = {}
            HM = NKC // 2

            def prefetch_mask(qt, part=None):
                if part is None:
                    prefetch_mask(qt, 0)
                    prefetch_mask(qt, 1)
                    return
                t = mtp.tile([P, HM, QT], BF16, name="mt", tag=f"mt{part}")
                mask_tiles[qt, part] = t
                sl = slice(qt * QT, (qt + 1) * QT)
                nc.sync.dma_start(
                    out=t,
                    in_=cpn(maskT_d[part * HM * P:(part + 1) * HM * P, sl]))

            prefetch_mask(0)

            # One flat software pipeline over all (qt, kc) chunks: QK/exp/mult
            # of chunk t are emitted together; P@V of chunk t-LOOK follows, so
            # qt boundaries interleave naturally and the PE never drains.
            sts, pts, pvs = {}, {}, {}

            def qk_expmul(t):
                qt, kc = divmod(t, NKC)
                st = ps_st.tile([P, QT], F32, name="st", tag="st")
                ksl = slice(kc * P, (kc + 1) * P)
                qsl = slice(qt * QT, (qt + 1) * QT)
                for hc in range(2):
                    nc.tensor.matmul(
                        st,
                        lhsT=kT_t[:, hc, ksl],
                        rhs=qT_t[:, hc, qsl],
                        start=(hc == 0), stop=(hc == 1),
                    )
                pe = pep.tile([P, QT], BF16, name="pe", tag="pe")
                nc.scalar.activation(
                    out=pe, in_=st, func=mybir.ActivationFunctionType.Exp,
                )
                pt = ptp.tile([P, QT], BF16, name="pt", tag="pt")
                nc.vector.tensor_tensor(
                    out=pt, in0=pe, in1=mask_tiles[qt, kc // HM][:, kc % HM, :],
                    op=mybir.AluOpType.mult,
                )
                pts[t] = pt

            def pv(t):
                qt, kc = divmod(t, NKC)
                if kc == 0:
                    pvs[qt] = [
                        ps_pv.tile([P, HV], F32, name="pv", tag=f"pv{j}")
                        for j in range(4)
                    ]
                pt = pts.pop(t)
                last = kc == NKC - 1
                ft = None
                for j in range(4):
                    nc.tensor.matmul(
                        pvs[qt][j],
                        lhsT=pt[:, j * P:(j + 1) * P],
                        rhs=v_t[:, kc, :],
                        start=(kc == 0), stop=last,
                    )
                    if last:
                        # finish column group j as soon as its accumulation
                        # ends: copies alternate ACT/DVE, DMA per pair
                        if j % 2 == 0:
                            ft = fin.tile([P, 2, HID + 1], F32, name="ft",
                                          tag=f"ft{j // 2}")
                            nc.scalar.copy(ft[:, 0, :], pvs[qt][j][:, :HID + 1])
                        else:
                            nc.vector.tensor_copy(ft[:, 1, :],
                                                  pvs[qt][j][:, :HID + 1])
                            r0 = qt * QT + (j - 1) * P
                            nc.sync.dma_start(
                                out=cpn(out_d[r0:r0 + 2 * P, :]), in_=ft)
                if last:
                    del pvs[qt], mask_tiles[qt, 0], mask_tiles[qt, 1]

            T = NQTT * NKC
            for t in range(T + LOOK):
                if t < T:
                    qt, kc = divmod(t, NKC)
                    if kc == 0:
                        if qt < NKT:
                            load_block(NKT + qt)
                        if qt < NQTT - 1:
                            prefetch_mask(qt + 1)
                    elif kc == 6 and qt < NKT:
                        emit_kq("q", 0, (NKT + qt) * QT)
                        emit_kq("q", 1, (NKT + qt) * QT)
                    qk_expmul(t)
                if t >= LOOK:
                    pv(t - LOOK)
    nc.finalize()
    return nc


def _get_nc():
    if "nc" not in _CACHE:
        _CACHE["nc"] = _build()
    return _CACHE["nc"]


def kernel(embedding, mask, Wq, bq, Wk, bk, Wv, bv):
    embedding = np.asarray(embedding, dtype=np.float32)
    mask = np.asarray(mask, dtype=np.float32)
    Wq = np.asarray(Wq, dtype=np.float32)
    Wk = np.asarray(Wk, dtype=np.float32)
    Wv = np.asarray(Wv, dtype=np.float32)
    bq = np.asarray(bq, dtype=np.float32)
    bk = np.asarray(bk, dtype=np.float32)
    bv = np.asarray(bv, dtype=np.float32)

    def pad_w(w, b, extra_one=False):
        wp = np.zeros((EMB + 1, HV if extra_one else HID), dtype=np.float32)
        wp[:EMB, :HID] = w
        wp[EMB, :HID] = b
        if extra_one:
            wp[EMB, HID] = 1.0
        return wp

    wq_p = pad_w(Wq, bq)
    wk_p = pad_w(Wk, bk)
    wv_p = pad_w(Wv, bv, extra_one=True)

    perms = [np.arange(L), np.concatenate([np.arange(KL, L), np.arange(KL)])]

    in_maps = []
    for c in range(NCORES):
        b, half = divmod(c, 2)
        perm = perms[half]
        embT = np.empty((EMB + 1, L), dtype=np.float32)
        embT[:EMB] = embedding[b].T[:, perm]
        embT[EMB] = 1.0
        ksl = slice(half * KL, (half + 1) * KL)
        mT = np.ascontiguousarray(mask[b].T[ksl][:, perm])
        # binary {1=masked} -> multiplicative {0=masked, 1=keep}
        mT = (1.0 - mT).astype(BF)
        in_maps.append({
            "embT": embT,
            "wq": wq_p, "wk": wk_p, "wv": wv_p,
            "maskT": mT,
        })

    nc = _get_nc()
    trace = bool(int(os.environ.get("BASS_KERNEL_TRACE", "0")))
    res = run_bass_kernel_spmd(nc, in_maps, core_ids=list(range(NCORES)), trace=trace)
    _CACHE["last_results"] = res

    full = np.empty((B, L, HID), dtype=np.float32)
    for b in range(B):
        r0 = res.results[2 * b]["out"].astype(np.float64)
        r1 = res.results[2 * b + 1]["out"].astype(np.float64)[perms[1]]
        num = r0[:, :HID] + r1[:, :HID]
        den = r0[:, HID:] + r1[:, HID:]
        full[b] = (num / den).astype(np.float32)
    return full
